# revision 11
# baseline (speedup 1.0000x reference)
"""Trainium2 Bass kernel for nn_AnswerModule (scatter_memory, 8 cores).

Strategy: pure data-parallel over batch (4 examples per core).  The
reference's heavy einsums W6@M / W7@M are algebraically collapsed via
matmul associativity: p1 = softmax((s@W6)@M), attn@W7b = p1@(M^T@W7b),
p2 = softmax((s@W7t + attn@W7b)@M).  The tiny GRU / alpha-attention
recurrence (O(B*D2) work) runs on host; everything touching M
(134 MB) runs on device, with M streamed HBM->SBUF exactly once per
core.  Softmax keeps n on partitions (no max subtraction - logits are
bounded ~60, exp-safe in f32) so all elementwise work uses 128 lanes.
Matmuls use float32r (full PE rate for moving dim >= 256).
"""

import sys

sys.path.insert(0, "/opt/trn_rl_repo")

import numpy as np

import concourse.bass as bass
import concourse.bacc as bacc
import concourse.mybir as mybir
from concourse import tile
from concourse.bass_utils import run_bass_kernel_spmd

B, QL, PL, T, D2 = 32, 64, 4096, 4, 256
NCORES = 8
BL = B // NCORES  # 4 examples per core
NCH = PL // 128  # 32 n-chunks
F32 = mybir.dt.float32
F32R = mybir.dt.float32r

_NC = None


def _r(ap):
    return ap


def _build_graph():
    nc = bacc.Bacc("TRN2", target_bir_lowering=False, debug=False)

    m_d = nc.dram_tensor("m", [BL, D2, PL], F32R, kind="ExternalInput").ap()
    r_d = nc.dram_tensor("r", [BL, D2, 260], F32R, kind="ExternalInput").ap()
    v1_d = nc.dram_tensor("v1", [BL, D2, T], F32, kind="ExternalInput").ap()
    eye_d = nc.dram_tensor("eye", [4, 4], F32, kind="ExternalInput").ap()
    ones_d = nc.dram_tensor("ones", [128, 128], F32R, kind="ExternalInput").ap()
    out_d = nc.dram_tensor("out", [BL, 2, NCH, 128], F32, kind="ExternalOutput").ap()

    AX = mybir.AxisListType.X
    ADD = mybir.AluOpType.add
    EXP = mybir.ActivationFunctionType.Exp
    LOG = getattr(mybir.ActivationFunctionType, "Log", None) or getattr(
        mybir.ActivationFunctionType, "Ln"
    )

    with tile.TileContext(nc) as tc:
        with (
            nc.allow_low_precision(reason="float32r is 4-byte, same width as f32"),
            tc.tile_pool(name="const", bufs=1) as cpool,
            tc.tile_pool(name="m", bufs=4) as mpool,
            tc.tile_pool(name="r", bufs=2) as rpool,
            tc.tile_pool(name="g", bufs=2) as gpool,
            tc.tile_pool(name="small", bufs=2) as spool,
            tc.tile_pool(name="keep", bufs=4) as kpool,
            tc.tile_pool(name="res", bufs=1) as respool,
            tc.tile_pool(name="ps1", bufs=2, space="PSUM") as ps1pool,
            tc.tile_pool(name="ps2", bufs=2, space="PSUM") as ps2pool,
            tc.tile_pool(name="psc", bufs=2, space="PSUM") as pscpool,
            tc.tile_pool(name="pss", bufs=2, space="PSUM") as psspool,
        ):
            ones_sb = cpool.tile([128, 128], F32R, tag="ones")
            nc.sync.dma_start(out=ones_sb[:], in_=ones_d[:, :])
            ones_col = ones_sb[:, 0:1]
            ones_row = ones_sb[0:1, :]
            eye_sb = cpool.tile([4, 4], F32, tag="eye")
            nc.sync.dma_start(out=eye_sb[:], in_=eye_d[:, :])
            res_sb = respool.tile([128, 2 * NCH * BL], F32, tag="res")
            lg_sb = respool.tile([128, 2 * NCH * BL], F32, tag="lg")

            def softmax_cols(expT, res_col):
                """expT: (128, NCH*4) unnormalized exp, n on partitions,
                col = nci*4 + t.  Writes sum_t expT*rz into res_sb[:, res_col:+NCH]
                and returns rz_row (1, T) sbuf tile of 1/Z."""
                psZ = psspool.tile([1, 128], F32, tag="pss")
                nc.tensor.matmul(
                    psZ[:], _r(ones_col), _r(expT[:]), start=True, stop=True
                )
                zrow = spool.tile([1, T], F32, tag="zrow")
                nc.vector.tensor_reduce(
                    zrow[:],
                    psZ[:].rearrange("p (n t) -> p t n", t=T),
                    axis=AX,
                    op=ADD,
                )
                rzrow = spool.tile([1, T], F32R, tag="rzrow")
                nc.vector.reciprocal(rzrow[:], zrow[:])
                psB = psspool.tile([128, T], F32, tag="pss")
                nc.tensor.matmul(
                    psB[:], _r(ones_row), _r(rzrow[:]), start=True, stop=True
                )
                rzb = spool.tile([128, T], F32, tag="rzb")
                nc.vector.tensor_copy(rzb[:], psB[:])
                prod = spool.tile([128, NCH * T], F32, tag="prod")
                try:
                    rzb_b = rzb[:].unsqueeze(1).broadcast_to((128, NCH, T))
                    nc.vector.tensor_mul(
                        prod[:].rearrange("p (n t) -> p n t", t=T),
                        expT[:].rearrange("p (n t) -> p n t", t=T),
                        rzb_b,
                    )
                    nc.vector.tensor_reduce(
                        res_sb[:, res_col : res_col + NCH],
                        prod[:].rearrange("p (n t) -> p n t", t=T),
                        axis=AX,
                        op=ADD,
                    )
                except Exception:
                    for i in range(NCH):
                        nc.vector.tensor_mul(
                            prod[:, i * T : (i + 1) * T],
                            expT[:, i * T : (i + 1) * T],
                            rzb[:],
                        )
                    nc.vector.tensor_reduce(
                        res_sb[:, res_col : res_col + NCH],
                        prod[:].rearrange("p (n t) -> p n t", t=T),
                        axis=AX,
                        op=ADD,
                    )
                return rzrow

            mds, v2ts = [], []
            for b in range(BL):
                md = []
                rt = []
                for dc in range(2):
                    mt = mpool.tile([128, PL], F32R, tag=f"m{dc}")
                    nc.gpsimd.dma_start(
                        out=mt[:], in_=m_d[b, dc * 128 : (dc + 1) * 128, :]
                    )
                    md.append(mt)
                    rr = rpool.tile([128, 260], F32R, tag=f"r{dc}")
                    nc.sync.dma_start(
                        out=rr[:], in_=r_d[b, dc * 128 : (dc + 1) * 128, :]
                    )
                    rt.append(rr)
                v1t = rpool.tile([128, 2 * T], F32, tag="v1t")
                for dc in range(2):
                    nc.sync.dma_start(
                        out=v1t[:, dc * T : (dc + 1) * T],
                        in_=v1_d[b, dc * 128 : (dc + 1) * 128, :],
                    )

                g_sb = gpool.tile([128, NCH * 256], F32R, tag="g")
                l1t = spool.tile([128, NCH * T], F32, tag="l1t")

                # pass 1: per n-chunk  [G | l1T] = M_chunk.T @ [W7b | SW6T]
                for i in range(NCH):
                    ps1 = ps1pool.tile([128, 260], F32, tag="ps1")
                    nc.tensor.matmul(
                        ps1[:],
                        _r(md[0][:, i * 128 : (i + 1) * 128]),
                        _r(rt[0][:]),
                        start=True,
                        stop=False,
                    )
                    nc.tensor.matmul(
                        ps1[:],
                        _r(md[1][:, i * 128 : (i + 1) * 128]),
                        _r(rt[1][:]),
                        start=False,
                        stop=True,
                    )
                    if i % 2 == 0:
                        nc.vector.tensor_copy(
                            g_sb[:, i * 256 : (i + 1) * 256], ps1[:, 0:256]
                        )
                    else:
                        nc.scalar.copy(
                            g_sb[:, i * 256 : (i + 1) * 256], ps1[:, 0:256]
                        )
                    nc.vector.tensor_copy(
                        l1t[:, i * T : (i + 1) * T], ps1[:, 256:260]
                    )

                expT = spool.tile([128, NCH * T], F32R, tag="expT")
                nc.scalar.activation(expT[:], l1t[:], EXP)

                rz1 = softmax_cols(expT, b * (2 * NCH))

                # C' = sum_n expT^T @ G   (4, 256) unnormalized attn@W7b
                psC = pscpool.tile([T, 256], F32, tag="psc")
                for i in range(NCH):
                    nc.tensor.matmul(
                        psC[:],
                        _r(expT[:, i * T : (i + 1) * T]),
                        _r(g_sb[:, i * 256 : (i + 1) * 256]),
                        start=(i == 0),
                        stop=(i == NCH - 1),
                    )
                # rz col (T,1) via outer-product trick
                psc4 = psspool.tile([T, 2], F32, tag="pss")
                nc.tensor.matmul(
                    psc4[:], _r(rz1[:]), _r(ones_sb[0:1, 0:2]), start=True, stop=True
                )
                rzcol = spool.tile([T, 1], F32, tag="rzcol")
                nc.vector.tensor_copy(rzcol[:], psc4[:, 0:1])
                cav = spool.tile([T, 256], F32, tag="cav")
                nc.vector.tensor_scalar_mul(cav[:], psC[:], rzcol[:])

                # v2T = transpose(cav) + v1T   -> (128, 2*T)
                v2t = kpool.tile([128, 2 * T], F32R, tag="v2t")
                for dc in range(2):
                    psT = psspool.tile([128, T], F32, tag="pss")
                    nc.tensor.transpose(
                        psT[:], cav[:, dc * 128 : (dc + 1) * 128], eye_sb[:]
                    )
                    nc.vector.tensor_add(
                        v2t[:, dc * T : (dc + 1) * T],
                        psT[:],
                        v1t[:, dc * T : (dc + 1) * T],
                    )

                mds.append(md)
                v2ts.append(v2t)

            for b in range(BL):
                md = mds[b]
                v2t = v2ts[b]
                # pass 2: l2T chunks = M_chunk.T @ v2
                l2t = spool.tile([128, NCH * T], F32, tag="l2t")
                for i in range(NCH):
                    ps2 = ps2pool.tile([128, T], F32, tag="ps2")
                    nc.tensor.matmul(
                        ps2[:],
                        _r(md[0][:, i * 128 : (i + 1) * 128]),
                        _r(v2t[:, 0:T]),
                        start=True,
                        stop=False,
                    )
                    nc.tensor.matmul(
                        ps2[:],
                        _r(md[1][:, i * 128 : (i + 1) * 128]),
                        _r(v2t[:, T : 2 * T]),
                        start=False,
                        stop=True,
                    )
                    nc.vector.tensor_copy(l2t[:, i * T : (i + 1) * T], ps2[:])

                exp2 = spool.tile([128, NCH * T], F32R, tag="exp2")
                nc.scalar.activation(exp2[:], l2t[:], EXP)
                softmax_cols(exp2, b * (2 * NCH) + NCH)

            # final: log(p/PL) over everything, one op + one DMA
            nc.scalar.activation(lg_sb[:], res_sb[:], LOG, scale=1.0 / PL)
            nc.sync.dma_start(
                out=out_d.rearrange("b o n p -> p (b o n)"), in_=lg_sb[:]
            )

    nc.compile()
    return nc


def _host_precompute(inp):
    H_q, M, W_4, W_6, W_7 = (
        inp["H_q"],
        inp["M"],
        inp["W_4"],
        inp["W_6"],
        inp["W_7"],
    )
    wih, whh, bih, bhh = (
        inp["gru_w_ih"],
        inp["gru_w_hh"],
        inp["gru_b_ih"],
        inp["gru_b_hh"],
    )
    lg = H_q @ W_4
    a = np.exp(lg - lg.max(1, keepdims=True))
    a /= a.sum(1, keepdims=True)
    s = np.einsum("bq,bqh->bh", a, H_q).astype(np.float32)
    x = M.mean(axis=2)
    gh = x @ whh.T + bhh
    ghr, ghz, ghn = np.split(gh, 3, axis=1)
    s_all = [s]
    for _ in range(T - 1):
        gi = s @ wih.T + bih
        gir, giz, gin = np.split(gi, 3, axis=1)
        r = 1.0 / (1.0 + np.exp(-(gir + ghr)))
        z = 1.0 / (1.0 + np.exp(-(giz + ghz)))
        n = np.tanh(gin + r * ghn)
        s = (1.0 - z) * n + z * x
        s_all.append(s)
    S = np.stack(s_all).astype(np.float32)  # (T, B, D2)
    SW6 = np.einsum("tbd,de->tbe", S, W_6).astype(np.float32)
    W7t, W7b = W_7[:D2], W_7[D2:]
    V1 = np.einsum("tbd,de->tbe", S, W7t).astype(np.float32)
    R = np.empty((B, D2, 260), np.float32)
    R[:, :, :256] = W7b[None]
    R[:, :, 256:] = SW6.transpose(1, 2, 0)  # (B, d, t)
    V1T = np.ascontiguousarray(V1.transpose(1, 2, 0))  # (B, d, t)
    return np.ascontiguousarray(R), V1T


def kernel(**inputs):
    global _NC
    inp = {
        k: np.ascontiguousarray(np.asarray(v, dtype=np.float32))
        for k, v in inputs.items()
    }
    R, V1T = _host_precompute(inp)
    M = inp["M"]
    eye4 = np.eye(4, dtype=np.float32)
    if _NC is None:
        _NC = _build_graph()
    in_maps = [
        {
            "m": np.ascontiguousarray(M[i * BL : (i + 1) * BL]),
            "r": np.ascontiguousarray(R[i * BL : (i + 1) * BL]),
            "v1": np.ascontiguousarray(V1T[i * BL : (i + 1) * BL]),
            "eye": eye4,
            "ones": np.ones((128, 128), np.float32),
        }
        for i in range(NCORES)
    ]
    global _LAST_IN_MAPS
    _LAST_IN_MAPS = in_maps
    res = run_bass_kernel_spmd(_NC, in_maps, core_ids=list(range(NCORES)))
    out1 = np.empty((B, PL), np.float32)
    out2 = np.empty((B, PL), np.float32)
    for i in range(NCORES):
        o = res.results[i]["out"]  # (BL, 2, NCH, 128)
        for b in range(BL):
            out1[i * BL + b] = o[b, 0].reshape(PL)
            out2[i * BL + b] = o[b, 1].reshape(PL)
    return out1, out2
